# revision 11
# baseline (speedup 1.0000x reference)
"""Trainium2 Bass kernel for nn_AttentionBlock (adaLN-modulated GroupNorm attention).

Sharding: data-parallel over batch B=8 -> one batch per NeuronCore (8 cores).
Each core runs the full block for its batch:
  groupnorm(32 groups) -> adaLN modulate -> qkv matmul -> 8-head attention
  (softmax over keys) -> proj matmul -> gated residual.

Layouts (per core):
  x, xm:      [C=512, T=1024] as 4 SBUF tiles [128, 1024]  (channels on partitions)
  qkv out:    12 tiles [128, 1024], channel order PERMUTED to type-major
              [q0..q7 | k0..k7 | v0..v7] (64 rows per head-type block) so that
              q_h and k_h always live at the same partition offset (0 or 64).
  scoresT:    [s, t] per head chunk-wise ([128 s, 512 t] PSUM), exp on ScalarE
              fuses the PSUM->SBUF eviction.
  PV:         U[65, t] = [vT | ones].T @ expT  -> row 64 is the softmax denom.
  normalize:  recip(denom) broadcast across partitions with a K=1 PE matmul.

All heavy matmuls run as float32r (full PE rate at N>=512, fp32 storage).
"""

import numpy as np

import concourse.bass as bass
import concourse.tile as tile
from concourse import bacc, mybir
from concourse.bass_utils import run_bass_kernel_spmd

AF = mybir.ActivationFunctionType
f32 = mybir.dt.float32
f32r = mybir.dt.float32r

B, C, HH, WW, E = 8, 512, 32, 32, 512
HEADS, G = 8, 32
T = HH * WW          # 1024
CH = C // HEADS      # 64
NC_ = C // 128       # 4 channel chunks
NO = 3 * C // 128    # 12 qkv output chunks
NT = T // 512        # 2 t-chunks of 512
NS = T // 128        # 8 s-chunks of 128
EPS = 1e-5

# 'f32r' = float32r matmuls (fast), 'f32' = exact fp32 (4x slower, for debug)
MM_DTYPE = "f32r"


def _perm():
    """new[512*ty + 64*h + r] = orig[192*h + 64*ty + r] (head-major -> type-major)."""
    p = np.empty(3 * C, np.int64)
    for h in range(HEADS):
        for ty in range(3):
            p[512 * ty + 64 * h : 512 * ty + 64 * h + 64] = (
                192 * h + 64 * ty + np.arange(64)
            )
    return p


def _build_program(mm_dtype=MM_DTYPE):
    nc = bacc.Bacc("TRN2", target_bir_lowering=False, debug=False, num_devices=8)

    mmdt = f32r if mm_dtype == "f32r" else f32

    def r(ap):
        return ap.bitcast(mmdt)

    # ---- DRAM parameters (per-core shards; weights replicated) ----
    x_d = nc.declare_dram_parameter("x", [C, T], f32, isOutput=False)
    emb_d = nc.declare_dram_parameter("emb", [E], f32, isOutput=False)
    qw_d = nc.declare_dram_parameter("qkv_wT", [C, 3 * C], f32, isOutput=False)
    qb_d = nc.declare_dram_parameter("qkv_b", [3 * C], f32, isOutput=False)
    aw_d = nc.declare_dram_parameter("ada_wT", [E, 3 * C], f32, isOutput=False)
    ab_d = nc.declare_dram_parameter("ada_b", [3 * C], f32, isOutput=False)
    pw_d = nc.declare_dram_parameter("proj_wT", [C, C], f32, isOutput=False)
    pb_d = nc.declare_dram_parameter("proj_b", [C], f32, isOutput=False)
    gind_d = nc.declare_dram_parameter("gind", [128, 8], f32, isOutput=False)
    gindT_d = nc.declare_dram_parameter("gindT", [8, 128], f32, isOutput=False)
    ident_d = nc.declare_dram_parameter("ident", [128, 128], f32, isOutput=False)
    ones_d = nc.declare_dram_parameter("ones", [128, 64], f32, isOutput=False)
    out_d = nc.declare_dram_parameter("out", [C, T], f32, isOutput=True)

    from contextlib import ExitStack

    with tile.TileContext(nc) as tc, ExitStack() as ctx:
        ctx.enter_context(
            nc.allow_low_precision(reason="float32r is fp32 storage; fp32 accumulate")
        )
        P = ctx.enter_context(tc.tile_pool(name="persist", bufs=1))
        EXPP = ctx.enter_context(tc.tile_pool(name="expp", bufs=6))
        ANP = ctx.enter_context(tc.tile_pool(name="anp", bufs=4))
        PSM = ctx.enter_context(tc.tile_pool(name="psm", bufs=3, space="PSUM"))
        PSU = ctx.enter_context(tc.tile_pool(name="psu", bufs=3, space="PSUM"))
        PSA = ctx.enter_context(tc.tile_pool(name="psa", bufs=2, space="PSUM"))

        # ---- persistent SBUF tiles + input DMAs ----
        gind_sb = P.tile([128, 8], f32, tag="gind")
        gindT_sb = P.tile([8, 128], f32, tag="gindT")
        ident_sb = P.tile([128, 128], f32, tag="ident")
        ones_sb = P.tile([128, 64], f32, tag="ones")
        emb_sb = P.tile([128, 4], f32, tag="emb")
        silu_sb = P.tile([128, 4], f32, tag="silu")
        qb_sb = P.tile([128, 12], f32, tag="qb")
        ab_sb = P.tile([128, 12], f32, tag="ab")
        pb_sb = P.tile([128, 4], f32, tag="pb")
        mod_sb = P.tile([128, 12], f32, tag="mod")

        nc.sync.dma_start(out=gind_sb, in_=gind_d.ap())
        nc.sync.dma_start(out=gindT_sb, in_=gindT_d.ap())
        nc.sync.dma_start(out=ident_sb, in_=ident_d.ap())
        nc.sync.dma_start(out=r(ones_sb[:]), in_=r(ones_d.ap()))
        nc.sync.dma_start(out=emb_sb, in_=emb_d.ap().rearrange("(f p) -> p f", p=128))
        nc.sync.dma_start(out=qb_sb, in_=qb_d.ap().rearrange("(f p) -> p f", p=128))
        nc.sync.dma_start(out=ab_sb, in_=ab_d.ap().rearrange("(f p) -> p f", p=128))
        nc.sync.dma_start(out=pb_sb, in_=pb_d.ap().rearrange("(f p) -> p f", p=128))

        xf = []
        for i in range(NC_):
            t_ = P.tile([128, T], f32, tag=f"xf{i}")
            nc.sync.dma_start(out=t_, in_=x_d.ap()[128 * i : 128 * (i + 1), :])
            xf.append(t_)
        aw = []
        for j in range(4):
            t_ = P.tile([128, 3 * C], f32, tag=f"aw{j}")
            nc.sync.dma_start(out=t_, in_=aw_d.ap()[128 * j : 128 * (j + 1), :])
            aw.append(t_)
        qw = []
        for j in range(4):
            t_ = P.tile([128, 3 * C], f32, tag=f"qw{j}")
            nc.sync.dma_start(
                out=r(t_[:]), in_=r(qw_d.ap()[128 * j : 128 * (j + 1), :])
            )
            qw.append(t_)
        pw = []
        for j in range(4):
            t_ = P.tile([128, C], f32, tag=f"pw{j}")
            nc.sync.dma_start(
                out=r(t_[:]), in_=r(pw_d.ap()[128 * j : 128 * (j + 1), :])
            )
            pw.append(t_)

        # ---- phase 1: adaLN modulation (PE) + groupnorm stats (DVE) ----
        sg_sb = P.tile([128, 4], f32, tag="sg")
        nc.scalar.activation(sg_sb, emb_sb, AF.Sigmoid)
        nc.vector.tensor_mul(silu_sb, emb_sb, sg_sb)
        mod_ps = PSA.tile([128, 12], f32, tag="aux")
        for m in range(NO):
            for j in range(4):
                nc.tensor.matmul(
                    mod_ps[:, m : m + 1],
                    aw[j][:, 128 * m : 128 * (m + 1)],
                    silu_sb[:, j : j + 1],
                    start=(j == 0),
                    stop=(j == 3),
                )
        nc.vector.tensor_add(mod_sb, mod_ps, ab_sb)

        mv = []
        for i in range(NC_):
            st6 = P.tile([128, 2, 6], f32, tag=f"st6{i}")
            xv = xf[i][:].rearrange("p (s f) -> p s f", f=512)
            for si in range(2):
                nc.vector.bn_stats(st6[:, si, :], xv[:, si, :])
            mv_i = P.tile([128, 2], f32, tag=f"mv{i}")
            nc.vector.bn_aggr(mv_i, st6)
            # E2 = var + mu^2 into col 1
            tm = P.tile([128, 1], f32, tag=f"tmu{i}")
            nc.vector.tensor_mul(tm, mv_i[:, 0:1], mv_i[:, 0:1])
            nc.vector.tensor_add(mv_i[:, 1:2], mv_i[:, 1:2], tm)
            mv.append(mv_i)

        stats8_ps = PSA.tile([8, 8], f32, tag="aux")
        for i in range(NC_):
            nc.tensor.matmul(
                stats8_ps[:, 2 * i : 2 * i + 2], gind_sb, mv[i], start=True, stop=True
            )
        s8 = P.tile([8, 8], f32, tag="s8")
        nc.vector.tensor_copy(s8, stats8_ps)
        musq8 = P.tile([8, 4], f32, tag="musq8")
        var8 = P.tile([8, 4], f32, tag="var8")
        sd8 = P.tile([8, 4], f32, tag="sd8")
        rstd8 = P.tile([8, 4], f32, tag="rstd8")
        for i in range(NC_):
            nc.vector.tensor_mul(
                musq8[:, i : i + 1], s8[:, 2 * i : 2 * i + 1], s8[:, 2 * i : 2 * i + 1]
            )
            nc.vector.tensor_sub(
                var8[:, i : i + 1], s8[:, 2 * i + 1 : 2 * i + 2], musq8[:, i : i + 1]
            )
        eps8 = P.tile([8, 1], f32, tag="eps8")
        nc.vector.memset(eps8, EPS)
        nc.scalar.activation(sd8, var8, AF.Sqrt, bias=eps8)
        nc.vector.reciprocal(rstd8, sd8)

        xm = []
        for i in range(NC_):
            statbc = PSA.tile([128, 2], f32, tag="aux")
            nc.tensor.matmul(
                statbc[:, 0:1], gindT_sb, s8[:, 2 * i : 2 * i + 1], start=True, stop=True
            )
            nc.tensor.matmul(
                statbc[:, 1:2], gindT_sb, rstd8[:, i : i + 1], start=True, stop=True
            )
            s1p = P.tile([128, 1], f32, tag=f"s1p{i}")
            A_i = P.tile([128, 1], f32, tag=f"A{i}")
            B_i = P.tile([128, 1], f32, tag=f"B{i}")
            tm2 = P.tile([128, 1], f32, tag=f"tm2{i}")
            nc.vector.tensor_scalar_add(s1p, mod_sb[:, 4 + i : 5 + i], 1.0)
            nc.vector.tensor_mul(A_i, statbc[:, 1:2], s1p)
            nc.vector.tensor_mul(tm2, statbc[:, 0:1], A_i)
            nc.vector.tensor_sub(B_i, mod_sb[:, i : i + 1], tm2)
            xm_i = P.tile([128, T], f32, tag=f"xm{i}")
            nc.scalar.activation(r(xm_i[:]), xf[i], AF.Identity, bias=B_i, scale=A_i)
            xm.append(xm_i)

        # ---- phase 2: qkv matmul [1536, 1024] (channel order = type-major) ----
        qkv = [P.tile([128, T], f32, tag=f"qkv{m}", name=f"qkv{m}") for m in range(NO)]
        for m in range(NO):
            for t in range(NT):
                ps = PSM.tile([128, 512], f32, tag="mm")
                for j in range(4):
                    nc.tensor.matmul(
                        ps,
                        r(qw[j][:, 128 * m : 128 * (m + 1)]),
                        r(xm[j][:, 512 * t : 512 * (t + 1)]),
                        start=(j == 0),
                        stop=(j == 3),
                    )
                dst = r(qkv[m][:, 512 * t : 512 * (t + 1)])
                if (m + t) % 2 == 0:
                    nc.scalar.activation(
                        dst, ps, AF.Identity, bias=qb_sb[:, m : m + 1], scale=1.0
                    )
                else:
                    nc.vector.tensor_scalar_add(dst, ps, qb_sb[:, m : m + 1])

        # ---- phase 3: v transposes (all heads up front) ----
        # head h: q = qkv[h//2][off:off+64], k = qkv[4+h//2][off], v = qkv[8+h//2][off]
        # with off = 64*(h%2)
        vT = [P.tile([128, 8, 65], f32, tag=f"vt{h}", name=f"vt{h}") for h in range(HEADS)]
        for h in range(HEADS):
            off = 64 * (h % 2)
            v_ap = qkv[8 + h // 2][off : off + 64, :]
            nc.vector.tensor_copy(
                r(vT[h][:, :, 64:65]),
                r(ones_sb[:, 0:8].rearrange("p (a o) -> p a o", o=1)),
            )
            for s in range(NS):
                vtr = PSA.tile([128, 64], f32, tag="aux")
                nc.tensor.transpose(
                    vtr,
                    v_ap[:, 128 * s : 128 * (s + 1)],
                    ident_sb[off : off + 64, off : off + 64],
                )
                nc.vector.tensor_copy(r(vT[h][:, s, 0:64]), vtr)

        # ---- phase 4: attention per head ----
        a_sb = [P.tile([128, T], f32, tag=f"asb{j}", name=f"asb{j}") for j in range(NC_)]
        for h in range(HEADS):
            off = 64 * (h % 2)
            q_ap = qkv[h // 2][off : off + 64, :]
            k_ap = qkv[4 + h // 2][off : off + 64, :]
            U = [PSU.tile([65, 512], f32, tag="u", name=f"u{h}_{t_}") for t_ in range(NT)]
            ex_tiles = {}
            for s in range(NS):
                for t in range(NT):
                    sc = PSM.tile([128, 512], f32, tag="mm")
                    nc.tensor.matmul(
                        sc,
                        r(k_ap[:, 128 * s : 128 * (s + 1)]),
                        r(q_ap[:, 512 * t : 512 * (t + 1)]),
                        start=True,
                        stop=True,
                    )
                    ex = EXPP.tile([128, 512], f32, tag="ex")
                    nc.scalar.activation(r(ex[:]), sc, AF.Exp, scale=0.125)
                    ex_tiles[(s, t)] = ex
                if s >= 1:
                    for t in range(NT):
                        nc.tensor.matmul(
                            U[t],
                            r(vT[h][:, s - 1, :]),
                            r(ex_tiles.pop((s - 1, t))),
                            start=(s - 1 == 0),
                            stop=False,
                        )
            for t in range(NT):
                nc.tensor.matmul(
                    U[t],
                    r(vT[h][:, NS - 1, :]),
                    r(ex_tiles.pop((NS - 1, t))),
                    start=False,
                    stop=True,
                )
            # normalize: a = U[0:64] * (1/denom), denom = U[64]
            for t in range(NT):
                rc = ANP.tile([65, 512], f32, tag="rc")
                nc.vector.reciprocal(r(rc[64:65, :]), U[t][64:65, :])
                rbc = PSA.tile([64, 512], f32, tag="aux")
                nc.tensor.matmul(
                    rbc, r(ones_sb[64:65, :]), r(rc[64:65, :]), start=True, stop=True
                )
                rbs = ANP.tile([64, 512], f32, tag="rbs")
                nc.scalar.activation(r(rbs[:]), rbc, AF.Copy)
                nc.vector.tensor_mul(r(rbs[:]), U[t][0:64, :], rbs)
                # place into a_sb at original head-major channel order (DMA shifts
                # partitions: head h -> rows 64*(h%2) of chunk h//2)
                nc.sync.dma_start(
                    out=r(a_sb[h // 2][off : off + 64, 512 * t : 512 * (t + 1)]),
                    in_=r(rbs[:]),
                )

        # ---- phase 5: proj + gated residual ----
        pbg = []
        for i_ in range(NC_):
            t_ = P.tile([128, 1], f32, tag=f"pbg{i_}")
            nc.vector.tensor_mul(t_, pb_sb[:, i_ : i_ + 1], mod_sb[:, 8 + i_ : 9 + i_])
            pbg.append(t_)
        for m in range(NC_):
            for t in range(NT):
                ps = PSM.tile([128, 512], f32, tag="mm")
                for j in range(4):
                    nc.tensor.matmul(
                        ps,
                        r(pw[j][:, 128 * m : 128 * (m + 1)]),
                        r(a_sb[j][:, 512 * t : 512 * (t + 1)]),
                        start=(j == 0),
                        stop=(j == 3),
                    )
                tg = ANP.tile([128, 512], f32, tag="tg")
                nc.scalar.activation(
                    tg, ps, AF.Identity, bias=pbg[m], scale=mod_sb[:, 8 + m : 9 + m]
                )
                # residual in-place into xf (xf never feeds a matmul)
                nc.vector.tensor_add(
                    xf[m][:, 512 * t : 512 * (t + 1)],
                    xf[m][:, 512 * t : 512 * (t + 1)],
                    tg,
                )
            nc.sync.dma_start(out=out_d.ap()[128 * m : 128 * (m + 1), :], in_=xf[m])

    nc.compile()
    return nc


_PROGRAM = None
LAST_RESULTS = None


def _get_program():
    global _PROGRAM
    if _PROGRAM is None:
        _PROGRAM = _build_program()
    return _PROGRAM


def kernel(x, emb, qkv_w, qkv_b, ada_w, ada_b, proj_w, proj_b, _trace=False):
    global LAST_RESULTS
    nc = _get_program()

    x = np.asarray(x, np.float32)
    emb = np.asarray(emb, np.float32)
    perm = _perm()
    qkv_wT = np.ascontiguousarray(np.asarray(qkv_w, np.float32)[perm, :].T)
    qkv_b_p = np.ascontiguousarray(np.asarray(qkv_b, np.float32)[perm])
    ada_wT = np.ascontiguousarray(np.asarray(ada_w, np.float32).T)
    ada_b = np.ascontiguousarray(np.asarray(ada_b, np.float32))
    proj_wT = np.ascontiguousarray(np.asarray(proj_w, np.float32).T)
    proj_b = np.ascontiguousarray(np.asarray(proj_b, np.float32))

    gind = np.repeat(np.eye(8, dtype=np.float32), 16, axis=0) / 16.0  # [128, 8]
    gindT = np.ascontiguousarray(np.repeat(np.eye(8, dtype=np.float32), 16, axis=0).T)
    ident = np.eye(128, dtype=np.float32)
    ones = np.ones((128, 64), dtype=np.float32)

    in_maps = []
    for b in range(B):
        in_maps.append(
            {
                "x": np.ascontiguousarray(x[b].reshape(C, T)),
                "emb": np.ascontiguousarray(emb[b]),
                "qkv_wT": qkv_wT,
                "qkv_b": qkv_b_p,
                "ada_wT": ada_wT,
                "ada_b": ada_b,
                "proj_wT": proj_wT,
                "proj_b": proj_b,
                "gind": gind,
                "gindT": gindT,
                "ident": ident,
                "ones": ones,
            }
        )

    res = run_bass_kernel_spmd(nc, in_maps, list(range(8)), trace=_trace)
    LAST_RESULTS = res
    out = np.stack([res.results[b]["out"] for b in range(B)], axis=0)
    return np.ascontiguousarray(out.reshape(B, C, HH, WW).astype(np.float32))


# revision 19
# speedup vs baseline: 1.4037x; 1.4037x over previous
"""Trainium2 Bass kernel for nn_AttentionBlock (adaLN-modulated GroupNorm attention).

Sharding: data-parallel over batch B=8 -> one batch per NeuronCore (8 cores).
Each core runs the full block for its batch:
  groupnorm(32 groups) -> adaLN modulate -> qkv matmul -> 8-head attention
  (softmax over keys) -> proj matmul -> gated residual.

Layouts (per core):
  x (fp32), xm (bf16):  [C=512, T=1024] as 4 tiles [128, 1024], channels on
                        partitions.
  qkv out (bf16): 12 tiles [128, 1024], channel order PERMUTED to type-major
              [q0..q7 | k0..k7 | v0..v7] (64 rows per head-type block) so that
              q_h and k_h always live at the same partition offset (0 or 64).
  scoresT:    [s, t] per head ([128 s, 1024 t] PSUM fp32), one batched exp on
              ScalarE fuses the PSUM->SBUF eviction (out bf16).
  PV:         U[65, t] = [vT | ones].T @ expT  -> row 64 is the softmax denom.
  normalize:  1/denom via reciprocal_approx_fast on partition 0, broadcast
              across partitions with gpsimd.partition_broadcast.

Matmuls run in bf16 (fp32 PSUM accumulation); groupnorm statistics stay fp32.
"""

import numpy as np

import concourse.bass as bass
import concourse.tile as tile
from concourse import bacc, mybir
from concourse.bass_utils import run_bass_kernel_spmd

AF = mybir.ActivationFunctionType
f32 = mybir.dt.float32
bf16 = mybir.dt.bfloat16

B, C, HH, WW, E = 8, 512, 32, 32, 512
HEADS, G = 8, 32
T = HH * WW          # 1024
CH = C // HEADS      # 64
NC_ = C // 128       # 4 channel chunks
NO = 3 * C // 128    # 12 qkv output chunks
NT = T // 512        # 2 t-chunks of 512
NS = T // 128        # 8 s-chunks of 128
EPS = 1e-5


def _perm():
    """new[512*ty + 64*h + r] = orig[192*h + 64*ty + r] (head-major -> type-major)."""
    p = np.empty(3 * C, np.int64)
    for h in range(HEADS):
        for ty in range(3):
            p[512 * ty + 64 * h : 512 * ty + 64 * h + 64] = (
                192 * h + 64 * ty + np.arange(64)
            )
    return p


def _build_program():
    nc = bacc.Bacc("TRN2", target_bir_lowering=False, debug=False, num_devices=8)

    # ---- DRAM parameters (per-core shards; weights replicated, bf16) ----
    x_d = nc.declare_dram_parameter("x", [C, T], f32, isOutput=False)
    emb_d = nc.declare_dram_parameter("emb", [E], f32, isOutput=False)
    qw_d = nc.declare_dram_parameter("qkv_wT", [C, 3 * C], bf16, isOutput=False)
    qb_d = nc.declare_dram_parameter("qkv_b", [3 * C], f32, isOutput=False)
    aw_d = nc.declare_dram_parameter("ada_wT", [E, 3 * C], bf16, isOutput=False)
    ab_d = nc.declare_dram_parameter("ada_b", [3 * C], f32, isOutput=False)
    pw_d = nc.declare_dram_parameter("proj_wT", [C, C], bf16, isOutput=False)
    pb_d = nc.declare_dram_parameter("proj_b", [C], f32, isOutput=False)
    gind_d = nc.declare_dram_parameter("gind", [128, 8], f32, isOutput=False)
    gindT_d = nc.declare_dram_parameter("gindT", [8, 128], f32, isOutput=False)
    ident_d = nc.declare_dram_parameter("ident", [128, 128], bf16, isOutput=False)
    ones_d = nc.declare_dram_parameter("ones", [128, 64], f32, isOutput=False)
    out_d = nc.declare_dram_parameter("out", [C, T], f32, isOutput=True)

    from contextlib import ExitStack

    with tile.TileContext(nc) as tc, ExitStack() as ctx:
        ctx.enter_context(
            nc.allow_low_precision(reason="bf16 matmul inputs; fp32 accumulate")
        )
        P = ctx.enter_context(tc.tile_pool(name="persist", bufs=1))
        # one shared PSUM tag: 2 rotating [128,1024] fp32 slots (4 banks)
        PSM = ctx.enter_context(tc.tile_pool(name="psm", bufs=2, space="PSUM"))
        PSU = ctx.enter_context(tc.tile_pool(name="psu", bufs=4, space="PSUM"))

        # ---- persistent SBUF tiles + input DMAs ----
        gind_sb = P.tile([128, 8], f32, tag="gind")
        gindT_sb = P.tile([8, 128], f32, tag="gindT")
        ident_sb = P.tile([128, 128], bf16, tag="ident")
        ones_sb = P.tile([128, 64], f32, tag="ones")
        emb_sb = P.tile([128, 4], f32, tag="emb")
        silu_sb = P.tile([128, 4], bf16, tag="silu")
        qb_sb = P.tile([128, 12], f32, tag="qb")
        ab_sb = P.tile([128, 12], f32, tag="ab")
        pb_sb = P.tile([128, 4], f32, tag="pb")
        mod_sb = P.tile([128, 12], f32, tag="mod")

        nc.sync.dma_start(out=gind_sb, in_=gind_d.ap())
        nc.sync.dma_start(out=gindT_sb, in_=gindT_d.ap())
        nc.sync.dma_start(out=ident_sb, in_=ident_d.ap())
        nc.sync.dma_start(out=ones_sb, in_=ones_d.ap())
        nc.sync.dma_start(out=emb_sb, in_=emb_d.ap().rearrange("(f p) -> p f", p=128))
        nc.sync.dma_start(out=qb_sb, in_=qb_d.ap().rearrange("(f p) -> p f", p=128))
        nc.sync.dma_start(out=ab_sb, in_=ab_d.ap().rearrange("(f p) -> p f", p=128))
        nc.sync.dma_start(out=pb_sb, in_=pb_d.ap().rearrange("(f p) -> p f", p=128))

        xf = []
        for i in range(NC_):
            t_ = P.tile([128, T], f32, tag=f"xf{i}")
            nc.sync.dma_start(out=t_, in_=x_d.ap()[128 * i : 128 * (i + 1), :])
            xf.append(t_)
        awp_cm = tc.tile_pool(name="awp", bufs=1)
        AWP = awp_cm.__enter__()
        aw = []
        for j in range(4):
            t_ = AWP.tile([128, 3 * C], bf16, tag=f"aw{j}", name=f"aw{j}")
            nc.sync.dma_start(out=t_, in_=aw_d.ap()[128 * j : 128 * (j + 1), :])
            aw.append(t_)
        qw = []
        for j in range(4):
            t_ = P.tile([128, 3 * C], bf16, tag=f"qw{j}")
            nc.sync.dma_start(out=t_, in_=qw_d.ap()[128 * j : 128 * (j + 1), :])
            qw.append(t_)
        pw = []
        for j in range(4):
            t_ = P.tile([128, C], bf16, tag=f"pw{j}")
            nc.sync.dma_start(out=t_, in_=pw_d.ap()[128 * j : 128 * (j + 1), :])
            pw.append(t_)

        # ---- phase 1: adaLN modulation (PE) + groupnorm stats (DVE) ----
        sg_sb = P.tile([128, 4], f32, tag="sg")
        nc.scalar.activation(sg_sb, emb_sb, AF.Sigmoid)
        nc.vector.tensor_mul(silu_sb, emb_sb, sg_sb)
        # mod^T = silu^T @ ada_wT as [1, 1536], then DRAM-bounce to [128, 12]
        mrow = P.tile([1, 3 * C], f32, tag="mrow")
        for oc in range(3):
            mps = PSM.tile([1, 512], f32, tag="sc", name=f"mps{oc}")
            for j in range(4):
                nc.tensor.matmul(
                    mps,
                    silu_sb[:, j : j + 1],
                    aw[j][:, 512 * oc : 512 * (oc + 1)],
                    start=(j == 0),
                    stop=(j == 3),
                )
            nc.vector.tensor_copy(mrow[:, 512 * oc : 512 * (oc + 1)], mps)
        awp_cm.__exit__(None, None, None)
        EXPP = ctx.enter_context(tc.tile_pool(name="expp", bufs=6))
        ANP = ctx.enter_context(tc.tile_pool(name="anp", bufs=4))
        modp_sb = P.tile([128, 12], f32, tag="modp")
        # partition-scatter via DRAM bounce (SBUF partition dim is physical)
        DP = ctx.enter_context(tc.tile_pool(name="dramp", bufs=1, space="DRAM"))
        mod_scr = DP.tile([1, 3 * C], f32, tag="modscr")
        nc.sync.dma_start(out=mod_scr, in_=mrow)
        nc.sync.dma_start(
            out=modp_sb, in_=mod_scr[0, :].rearrange("(f p) -> p f", p=128)
        )
        nc.vector.tensor_add(mod_sb, modp_sb, ab_sb)

        mv = []
        for i in range(NC_):
            st6 = P.tile([128, 2, 6], f32, tag=f"st6{i}")
            xv = xf[i][:].rearrange("p (s f) -> p s f", f=512)
            for si in range(2):
                nc.vector.bn_stats(st6[:, si, :], xv[:, si, :])
            mv_i = P.tile([128, 2], f32, tag=f"mv{i}")
            nc.vector.bn_aggr(mv_i, st6)
            # E2 = var + mu^2 into col 1
            tm = P.tile([128, 1], f32, tag=f"tmu{i}")
            nc.vector.tensor_mul(tm, mv_i[:, 0:1], mv_i[:, 0:1])
            nc.vector.tensor_add(mv_i[:, 1:2], mv_i[:, 1:2], tm)
            mv.append(mv_i)

        stats8_ps = PSM.tile([8, 8], f32, tag="sc", name="stats8")
        for i in range(NC_):
            nc.tensor.matmul(
                stats8_ps[:, 2 * i : 2 * i + 2], gind_sb, mv[i], start=True, stop=True
            )
        s8 = P.tile([8, 8], f32, tag="s8")
        nc.vector.tensor_copy(s8, stats8_ps)
        musq8 = P.tile([8, 4], f32, tag="musq8")
        var8 = P.tile([8, 4], f32, tag="var8")
        sd8 = P.tile([8, 4], f32, tag="sd8")
        rstd8 = P.tile([8, 4], f32, tag="rstd8")
        for i in range(NC_):
            nc.vector.tensor_mul(
                musq8[:, i : i + 1], s8[:, 2 * i : 2 * i + 1], s8[:, 2 * i : 2 * i + 1]
            )
            nc.vector.tensor_sub(
                var8[:, i : i + 1], s8[:, 2 * i + 1 : 2 * i + 2], musq8[:, i : i + 1]
            )
        eps8 = P.tile([8, 1], f32, tag="eps8")
        nc.vector.memset(eps8, EPS)
        nc.scalar.activation(sd8, var8, AF.Sqrt, bias=eps8)
        nc.vector.reciprocal(rstd8, sd8)

        xm = []
        for i in range(NC_):
            statbc = PSM.tile([128, 2], f32, tag="sc", name=f"statbc{i}")
            nc.tensor.matmul(
                statbc[:, 0:1], gindT_sb, s8[:, 2 * i : 2 * i + 1], start=True, stop=True
            )
            nc.tensor.matmul(
                statbc[:, 1:2], gindT_sb, rstd8[:, i : i + 1], start=True, stop=True
            )
            s1p = P.tile([128, 1], f32, tag=f"s1p{i}")
            A_i = P.tile([128, 1], f32, tag=f"A{i}")
            B_i = P.tile([128, 1], f32, tag=f"B{i}")
            tm2 = P.tile([128, 1], f32, tag=f"tm2{i}")
            nc.vector.tensor_scalar_add(s1p, mod_sb[:, 4 + i : 5 + i], 1.0)
            nc.vector.tensor_mul(A_i, statbc[:, 1:2], s1p)
            nc.vector.tensor_mul(tm2, statbc[:, 0:1], A_i)
            nc.vector.tensor_sub(B_i, mod_sb[:, i : i + 1], tm2)
            xm_i = P.tile([128, T], bf16, tag=f"xm{i}")
            nc.scalar.activation(xm_i, xf[i], AF.Identity, bias=B_i, scale=A_i)
            xm.append(xm_i)

        # ---- phase 2: qkv matmul [1536, 1024] (channel order = type-major) ----
        qkv = [P.tile([128, T], bf16, tag=f"qkv{m}", name=f"qkv{m}") for m in range(NO)]
        # chunk order: all three chunks of head pair 0 first, then pair 1, ...
        m_order = [p + 4 * ty for p in range(4) for ty in range(3)]
        for m in m_order:
            ps = PSM.tile([128, T], f32, tag="sc", name=f"qkvps{m}")
            for t in range(NT):
                for j in range(4):
                    nc.tensor.matmul(
                        ps[:, 512 * t : 512 * (t + 1)],
                        qw[j][:, 128 * m : 128 * (m + 1)],
                        xm[j][:, 512 * t : 512 * (t + 1)],
                        start=(j == 0),
                        stop=(j == 3),
                    )
            nc.vector.tensor_scalar_add(qkv[m][:], ps, qb_sb[:, m : m + 1])

        # ---- phase 3+4: attention, head pairs interleaved ----
        # Heads 2j / 2j+1 live at partition offsets 0 / 64 of the same qkv
        # tiles; interleaving their K=64 matmuls puts them in different PE
        # row-groups so they can execute concurrently.
        a_sb = [
            P.tile([128, T], bf16, tag=f"asb{j}", name=f"asb{j}") for j in range(NC_)
        ]
        vT = [
            P.tile([128, 8, 65], bf16, tag=f"vt{h}", name=f"vt{h}")
            for h in range(HEADS)
        ]
        for hp in range(4):
            heads = (2 * hp, 2 * hp + 1)
            for h in heads:
                nc.vector.tensor_copy(
                    vT[h][:, :, 64:65],
                    ones_sb[:, 0:8].rearrange("p (a o) -> p a o", o=1),
                )
            for s in range(NS):
                for h in heads:
                    off = 64 * (h % 2)
                    v_ap = qkv[8 + h // 2][off : off + 64, :]
                    vtr = PSM.tile([128, 64], bf16, tag="sc", name=f"vtr{hp}_{s}_{h}")
                    nc.tensor.transpose(
                        vtr,
                        v_ap[:, 128 * s : 128 * (s + 1)],
                        ident_sb[off : off + 64, off : off + 64],
                    )
                    nc.vector.tensor_copy(vT[h][:, s, 0:64], vtr)
            U = {}
            for h in heads:
                for t in range(NT):
                    U[(h, t)] = PSU.tile([65, 512], f32, tag="u", name=f"u{h}_{t}")
            ex_tiles = {}
            for s in range(NS):
                for h in heads:
                    off = 64 * (h % 2)
                    q_ap = qkv[h // 2][off : off + 64, :]
                    k_ap = qkv[4 + h // 2][off : off + 64, :]
                    sc = PSM.tile([128, T], f32, tag="sc", name=f"sc{hp}_{s}_{h}")
                    for t in range(NT):
                        nc.tensor.matmul(
                            sc[:, 512 * t : 512 * (t + 1)],
                            k_ap[:, 128 * s : 128 * (s + 1)],
                            q_ap[:, 512 * t : 512 * (t + 1)],
                            start=True,
                            stop=True,
                        )
                    ex = EXPP.tile([128, T], bf16, tag="ex")
                    nc.scalar.activation(ex, sc, AF.Exp, scale=0.125)
                    ex_tiles[(h, s)] = ex
                if s >= 1:
                    for h in heads:
                        ex = ex_tiles.pop((h, s - 1))
                        for t in range(NT):
                            nc.tensor.matmul(
                                U[(h, t)],
                                vT[h][:, s - 1, :],
                                ex[:, 512 * t : 512 * (t + 1)],
                                start=(s - 1 == 0),
                                stop=False,
                            )
            for h in heads:
                ex = ex_tiles.pop((h, NS - 1))
                for t in range(NT):
                    nc.tensor.matmul(
                        U[(h, t)],
                        vT[h][:, NS - 1, :],
                        ex[:, 512 * t : 512 * (t + 1)],
                        start=False,
                        stop=True,
                    )
            # normalize: a = U[0:64] / denom (denom = row 64); the reciprocal
            # runs on partition 0 (partition_broadcast sources partition 0)
            for h in heads:
                off = 64 * (h % 2)
                for t in range(NT):
                    rc = ANP.tile([65, 512], f32, tag="rc", bufs=2)
                    nc.vector.tensor_copy(rc[64:65, :], U[(h, t)][64:65, :])
                    rc0 = ANP.tile([1, 512], f32, tag="rc0", bufs=2)
                    nc.sync.dma_start(out=rc0, in_=rc[64:65, :])
                    nc.vector.reciprocal_approx_fast(out=rc0[:], in_=rc0[:])
                    rbs = ANP.tile([64, 512], f32, tag="rbs")
                    nc.gpsimd.partition_broadcast(rbs[:], rc0[:])
                    abf = ANP.tile([64, 512], bf16, tag="abf")
                    nc.vector.tensor_mul(abf, U[(h, t)][0:64, :], rbs)
                    nc.sync.dma_start(
                        out=a_sb[h // 2][off : off + 64, 512 * t : 512 * (t + 1)],
                        in_=abf,
                    )

        # ---- phase 5: proj + gated residual ----
        pbg = []
        for i_ in range(NC_):
            t_ = P.tile([128, 1], f32, tag=f"pbg{i_}")
            nc.vector.tensor_mul(t_, pb_sb[:, i_ : i_ + 1], mod_sb[:, 8 + i_ : 9 + i_])
            pbg.append(t_)
        for m in range(NC_):
            ps = PSM.tile([128, T], f32, tag="sc", name=f"projps{m}")
            for t in range(NT):
                for j in range(4):
                    nc.tensor.matmul(
                        ps[:, 512 * t : 512 * (t + 1)],
                        pw[j][:, 128 * m : 128 * (m + 1)],
                        a_sb[j][:, 512 * t : 512 * (t + 1)],
                        start=(j == 0),
                        stop=(j == 3),
                    )
            tg = ANP.tile([128, T], f32, tag="tg", bufs=2)
            nc.scalar.activation(
                tg, ps, AF.Identity, bias=pbg[m], scale=mod_sb[:, 8 + m : 9 + m]
            )
            # residual in-place into xf (xf never feeds a matmul)
            nc.vector.tensor_add(xf[m][:], xf[m][:], tg)
            nc.sync.dma_start(out=out_d.ap()[128 * m : 128 * (m + 1), :], in_=xf[m])

    nc.compile()
    return nc


_PROGRAM = None
LAST_RESULTS = None


def _get_program():
    global _PROGRAM
    if _PROGRAM is None:
        _PROGRAM = _build_program()
    return _PROGRAM


def kernel(x, emb, qkv_w, qkv_b, ada_w, ada_b, proj_w, proj_b, _trace=False):
    global LAST_RESULTS
    import ml_dtypes

    nc = _get_program()

    x = np.asarray(x, np.float32)
    emb = np.asarray(emb, np.float32)
    perm = _perm()
    bf = ml_dtypes.bfloat16
    qkv_wT = np.ascontiguousarray(np.asarray(qkv_w, np.float32)[perm, :].T.astype(bf))
    qkv_b_p = np.ascontiguousarray(np.asarray(qkv_b, np.float32)[perm])
    ada_wT = np.ascontiguousarray(np.asarray(ada_w, np.float32).T.astype(bf))
    ada_b = np.ascontiguousarray(np.asarray(ada_b, np.float32))
    proj_wT = np.ascontiguousarray(np.asarray(proj_w, np.float32).T.astype(bf))
    proj_b = np.ascontiguousarray(np.asarray(proj_b, np.float32))

    gind = np.repeat(np.eye(8, dtype=np.float32), 16, axis=0) / 16.0  # [128, 8]
    gindT = np.ascontiguousarray(np.repeat(np.eye(8, dtype=np.float32), 16, axis=0).T)
    ident = np.eye(128, dtype=bf)
    ones = np.ones((128, 64), dtype=np.float32)

    in_maps = []
    for b in range(B):
        in_maps.append(
            {
                "x": np.ascontiguousarray(x[b].reshape(C, T)),
                "emb": np.ascontiguousarray(emb[b]),
                "qkv_wT": qkv_wT,
                "qkv_b": qkv_b_p,
                "ada_wT": ada_wT,
                "ada_b": ada_b,
                "proj_wT": proj_wT,
                "proj_b": proj_b,
                "gind": gind,
                "gindT": gindT,
                "ident": ident,
                "ones": ones,
            }
        )

    res = run_bass_kernel_spmd(nc, in_maps, list(range(8)), trace=_trace)
    LAST_RESULTS = res
    out = np.stack([res.results[b]["out"] for b in range(B)], axis=0)
    return np.ascontiguousarray(out.reshape(B, C, HH, WW).astype(np.float32))
